# revision 1
# baseline (speedup 1.0000x reference)
"""Trainium2 Bass kernel for the KAN layer (nn_KANLayer).

Math restructure
----------------
Reference computes, for x in [0,1) on a uniform extended B-spline grid
(g0 = grid[0,0], h = grid spacing, t = (x-g0)/h in [7,11), t' = t-9):

  y[b,o] = sum_i mask[i,o]*(scale_base[i,o]*silu(x[b,i])
                            + scale_sp[i,o]*sum_k basis_k(x[b,i])*coef[i,o,k])

On the restricted domain every cubic B-spline basis function is an exact
linear combination of 8 fixed functions of x:
  phi = [1, t', t'^2, t'^3, relu(t'+1)^3, relu(t')^3, relu(t'-1)^3]  (+ silu)
so the whole layer collapses to one matmul with host-folded weights:
  y = F(x) @ W_fold + bias,   F: (B, I*7),  W_fold: (I*7, O)

Sharding: out_dim split x4, batch split x2  ->  8 cores, no collectives.
Each core: compute its feature planes (DVE/ACT, fp16), one 28-chunk
accumulated matmul (PE, fp16 inputs / fp32 PSUM), bias add, store.

Host does only weight folding (offline-style weight prep), slicing and
layout swizzles; all per-token math (features, silu, matmul) runs on
device.
"""

import sys

for _p in ("/opt/trn_rl_repo", "/opt/trn_rl_repo/concourse"):
    if _p not in sys.path:
        sys.path.insert(0, _p)

import numpy as np

import concourse.bass as bass
import concourse.bacc as bacc
import concourse.mybir as mybir
import concourse.tile as tile
from concourse.bass_utils import run_bass_kernel_spmd


def _install_ntff_hook_shim():
    """antenv in this image lacks axon_hooks; bass_utils imports it whenever
    tracing is requested (including via BASS_TRACE env). Provide the
    documented ctypes-based hook so that path works instead of crashing."""
    try:
        import antenv.axon_hooks  # noqa: F401
        return
    except ImportError:
        pass
    import types, contextlib, ctypes, os

    so_path = "/opt/axon/libaxon_pjrt.so"
    hook = None
    if os.path.exists(so_path):
        try:
            lib = ctypes.CDLL(so_path)
            if hasattr(lib, "axon_start_nrt_profile"):
                lib.axon_start_nrt_profile.argtypes = [
                    ctypes.POINTER(ctypes.c_int64), ctypes.c_size_t]
                lib.axon_start_nrt_profile.restype = ctypes.c_int64
                lib.axon_stop_nrt_profile.argtypes = [ctypes.c_char_p]
                lib.axon_stop_nrt_profile.restype = ctypes.c_int64

                @contextlib.contextmanager
                def _hook(output_dir, device_ids):
                    import jax
                    jax.devices()
                    if device_ids:
                        ids = (ctypes.c_int64 * len(device_ids))(*device_ids)
                        rc = lib.axon_start_nrt_profile(ids, len(device_ids))
                    else:
                        rc = lib.axon_start_nrt_profile(None, 0)
                    if rc != 0:
                        raise RuntimeError(f"axon_start_nrt_profile rc={rc}")
                    try:
                        yield
                    finally:
                        n = lib.axon_stop_nrt_profile(str(output_dir).encode())
                        print(f"ntff profile: {n} file(s) in {output_dir}")

                hook = _hook
        except OSError:
            pass

    try:
        import antenv
    except ImportError:
        return
    m = types.ModuleType("antenv.axon_hooks")
    m.get_axon_ntff_profile_hook = (lambda h: (lambda: h))(hook)
    m.set_axon_ntff_profile_hook = lambda h: None
    sys.modules["antenv.axon_hooks"] = m
    antenv.axon_hooks = m


_install_ntff_hook_shim()

B, I, O, NUM, K = 512, 512, 512, 8, 3
NPLANES = 7          # t', t'^2, t'^3, r8^3, r9^3, r10^3, silu
O_SPLIT, B_SPLIT = 4, 2
OQ = O // O_SPLIT    # 128 out dims per core
BH = B // B_SPLIT    # 256 batch rows per core
ICHUNKS = I // 128   # 4 partition chunks of the in_dim
FREE = ICHUNKS * BH  # 1024: feature-plane free dim (i-chunks stacked)
NCORES = O_SPLIT * B_SPLIT

F32 = mybir.dt.float32
F16 = mybir.dt.float16


def _basis_coeffs(g0, h):
    """Exact expansion of basis_k (k=0..NUM+K-1) in the phi basis.

    basis_k(x) = N(t - k) with N the cardinal cubic B-spline
    N(s) = sum_j (-1)^j C(4,j)/6 * relu(s-j)^3.  For t in [7,11) the knots
    at p <= 7 are always active (pure cubics -> poly part around t'=t-9)
    and knots p in {8,9,10} stay as relu kinks; p >= 11 never activates.
    Returns C (8, NUM+K): rows = [1, t', t'^2, t'^3, r8^3, r9^3, r10^3].
    """
    from math import comb

    nb = NUM + K
    C = np.zeros((7, nb))
    for k in range(nb):
        for j in range(5):
            w = ((-1) ** j) * comb(4, j) / 6.0
            p = k + j                      # knot index: relu(t - p)^3
            if p >= 11:
                continue
            if p <= 7:
                # always-active: (t - p)^3 = (t' + (9 - p))^3, expand
                c = 9.0 - p
                C[0, k] += w * c ** 3
                C[1, k] += w * 3 * c ** 2
                C[2, k] += w * 3 * c
                C[3, k] += w
            else:
                C[4 + (p - 8), k] += w
    return C


def _fold_weights(grid, coef, scale_base, scale_sp, mask):
    g0 = float(grid[0, 0])
    h = float(grid[0, 1]) - g0
    C = _basis_coeffs(g0, h)                                   # (7, 11)
    A = (mask.astype(np.float64) * scale_sp.astype(np.float64))[:, :, None] \
        * coef.astype(np.float64)                              # (I, O, 11)
    Wf = np.einsum("fk,iok->fio", C[1:7], A)                   # (6, I, O)
    W_silu = (mask.astype(np.float64) * scale_base.astype(np.float64))[None]
    W_all = np.concatenate([Wf, W_silu], axis=0)               # (7, I, O)
    bias = np.einsum("k,iok->o", C[0], A)                      # (O,)
    a1 = 1.0 / h                                               # t' = a1*x + a0
    a0 = -g0 / h - 9.0
    return W_all, bias, a1, a0


def _build_nc(a1, a0):
    AF = mybir.ActivationFunctionType
    AO = mybir.AluOpType

    nc = bacc.Bacc("TRN2", target_bir_lowering=False, debug=False)
    xt_d = nc.dram_tensor("xt", [128, FREE], F32, kind="ExternalInput").ap()
    w_d = nc.dram_tensor("w", [128, NPLANES * I], F16, kind="ExternalInput").ap()
    b_d = nc.dram_tensor("bias", [128, 1], F32, kind="ExternalInput").ap()
    o_d = nc.dram_tensor("out", [128, BH], F32, kind="ExternalOutput").ap()

    with tile.TileContext(nc) as tc:
        with (
            tc.tile_pool(name="main", bufs=1) as pool,
            tc.tile_pool(name="ps", bufs=1, space=bass.MemorySpace.PSUM) as pp,
        ):
            # xs on the SP HWDGE ring, weights on the ACT HWDGE ring so the
            # two loads run in parallel instead of serializing on one ring
            xs = pool.tile([128, FREE], F32, tag="xs")
            nc.sync.dma_start(xs[:], xt_d[:])
            w_sb = pool.tile([128, NPLANES * I], F16, tag="w")
            for f in range(NPLANES):
                nc.sync.dma_start(
                    w_sb[:, f * I:(f + 1) * I], w_d[:, f * I:(f + 1) * I]
                )
            bias_sb = pool.tile([128, 1], F32, tag="bias")
            nc.sync.dma_start(bias_sb[:], b_d[:])

            planes = [
                pool.tile([128, FREE], F16, tag=f"pl{j}", name=f"pl{j}")
                for j in range(NPLANES)
            ]
            tp, p2, p3, f4, f5, f6, sil = planes
            a8 = pool.tile([128, FREE], F16, tag="a8")
            a10 = pool.tile([128, FREE], F16, tag="a10")
            s8 = pool.tile([128, FREE], F16, tag="s8")
            s10 = pool.tile([128, FREE], F16, tag="s10")

            zeroc = pool.tile([128, 1], F32, tag="zeroc", name="zeroc")
            b2c = pool.tile([128, 1], F32, tag="b2c", name="b2c")
            b8c = pool.tile([128, 1], F32, tag="b8c", name="b8c")
            b10c = pool.tile([128, 1], F32, tag="b10c", name="b10c")
            nc.vector.memset(zeroc[:], 0.0)
            nc.vector.memset(b2c[:], a0)
            nc.vector.memset(b8c[:], a0 + 1.0)
            nc.vector.memset(b10c[:], a0 - 1.0)
            sg = pool.tile([128, FREE], F16, tag="sg", name="sg")

            # ACT: sigmoid + the three shifted squares; DVE: t', relus, products
            nc.vector.tensor_scalar(tp[:], xs[:], a1, a0, AO.mult, AO.add)
            nc.scalar.activation(sg[:], xs[:], AF.Sigmoid, bias=zeroc[:])
            nc.scalar.activation(p2[:], xs[:], AF.Square, bias=b2c[:], scale=a1)
            nc.scalar.activation(s8[:], xs[:], AF.Square, bias=b8c[:], scale=a1)
            nc.scalar.activation(s10[:], xs[:], AF.Square, bias=b10c[:], scale=a1)
            nc.vector.tensor_scalar(a8[:], tp[:], 1.0, 0.0, AO.add, AO.max)
            nc.vector.tensor_scalar(a10[:], tp[:], -1.0, 0.0, AO.add, AO.max)
            nc.vector.tensor_mul(p3[:], p2[:], tp[:])
            nc.vector.tensor_mul(f4[:], s8[:], a8[:])
            nc.vector.scalar_tensor_tensor(f5[:], tp[:], 0.0, p2[:], AO.max, AO.mult)
            nc.vector.tensor_mul(f6[:], s10[:], a10[:])
            # silu = x * sigmoid(x)
            nc.vector.scalar_tensor_tensor(sil[:], sg[:], 1.0, xs[:], AO.mult, AO.mult)

            acc = pp.tile([128, BH], F32, tag="acc")
            # matmul chunks ordered by plane readiness
            order = [0, 6, 1, 2, 4, 3, 5]
            n = 0
            for f in order:
                for ic in range(ICHUNKS):
                    c = f * ICHUNKS + ic
                    nc.tensor.matmul(
                        acc[:],
                        w_sb[:, c * 128:(c + 1) * 128],
                        planes[f][:, ic * BH:(ic + 1) * BH],
                        start=(n == 0),
                        stop=(n == NPLANES * ICHUNKS - 1),
                    )
                    n += 1

            outs = pool.tile([128, BH], F32, tag="outs")
            nc.vector.tensor_scalar(outs[:], acc[:], bias_sb[:, 0:1], None, AO.add)
            nc.sync.dma_start(o_d[:], outs[:])

    nc.compile()
    return nc


def _make_in_maps(x, W_all, bias):
    """Slice + layout-swizzle the folded weights and x for the 8 cores."""
    in_maps = []
    for c in range(NCORES):
        oq, bh = c // B_SPLIT, c % B_SPLIT
        xs = x[bh * BH:(bh + 1) * BH, :]                       # (BH, I)
        xt = np.ascontiguousarray(
            xs.T.reshape(ICHUNKS, 128, BH).transpose(1, 0, 2).reshape(128, FREE)
        ).astype(np.float32)
        Wq = W_all[:, :, oq * OQ:(oq + 1) * OQ]                # (7, I, OQ)
        w = np.ascontiguousarray(
            Wq.reshape(NPLANES, ICHUNKS, 128, OQ)
            .transpose(2, 0, 1, 3)
            .reshape(128, NPLANES * I)
        ).astype(np.float16)
        b = np.ascontiguousarray(
            bias[oq * OQ:(oq + 1) * OQ, None]
        ).astype(np.float32)
        in_maps.append({"xt": xt, "w": w, "bias": b})
    return in_maps


def _assemble(results):
    full = np.empty((B, O), np.float32)
    for c in range(NCORES):
        oq, bh = c // B_SPLIT, c % B_SPLIT
        full[bh * BH:(bh + 1) * BH, oq * OQ:(oq + 1) * OQ] = results[c]["out"].T
    return full


_CACHED = {}


def _get_nc(a1, a0):
    key = (a1, a0)
    if key not in _CACHED:
        _CACHED[key] = _build_nc(a1, a0)
    return _CACHED[key]


def kernel(x, grid, coef, scale_base, scale_sp, mask, _run_kwargs=None):
    x = np.asarray(x)
    W_all, bias, a1, a0 = _fold_weights(
        np.asarray(grid), np.asarray(coef), np.asarray(scale_base),
        np.asarray(scale_sp), np.asarray(mask)
    )
    nc = _get_nc(a1, a0)
    in_maps = _make_in_maps(x, W_all, bias)
    res = run_bass_kernel_spmd(
        nc, in_maps, core_ids=list(range(NCORES)), **(_run_kwargs or {})
    )
    out = _assemble(res.results)
    if _run_kwargs:
        kernel.last_result = res
    return out



# revision 5
# speedup vs baseline: 1.1674x; 1.1674x over previous
"""Trainium2 Bass kernel for the KAN layer (nn_KANLayer).

Math restructure
----------------
Reference computes, for x in [0,1) on a uniform extended B-spline grid
(g0 = grid[0,0], h = grid spacing, t = (x-g0)/h - 9 in [-2,2)):

  y[b,o] = sum_i mask[i,o]*(scale_base[i,o]*silu(x[b,i])
                            + scale_sp[i,o]*sum_k basis_k(x[b,i])*coef[i,o,k])

On the restricted domain every cubic B-spline basis function is an exact
linear combination of 8 fixed functions of x, so the layer collapses to
one matmul with host-folded weights.  Device feature planes (fp16):

  P0 = t              (DVE tensor_scalar)
  P1 = t^2            (ACT Square)
  P2 = t^3            (DVE t*t^2)
  P3 = |t^3|          (ACT Abs)     [relu(t)^3 = (t^3+|t^3|)/2, host-folded]
  P4 = relu(t+1)^3    (DVE (t+1)^2 * relu(t+1), square on ACT)
  P5 = relu(t-1)^3    (DVE, same)
  P6 = silu(x)        (ACT Silu)

The per-output bias is folded into the matmul as a 29th weight chunk
against an all-ones plane.  y = F(x) @ W_fold, 29 accumulated matmuls.

Sharding: out_dim split x4, batch split x2 -> 8 cores, no collectives.
All inputs fp16; weights ship as one DRAM tensor split into 3 DMA pieces
ordered to match plane readiness; dummy warm-up matmuls keep the PE HAM
un-throttled during the input DMA window; ACT does the final PSUM->SBUF
copy; output ships fp16 and is cast on host.
"""

import sys

for _p in ("/opt/trn_rl_repo", "/opt/trn_rl_repo/concourse"):
    if _p not in sys.path:
        sys.path.insert(0, _p)

import numpy as np

import concourse.bass as bass
import concourse.bacc as bacc
import concourse.mybir as mybir
import concourse.tile as tile
from concourse.bass_utils import run_bass_kernel_spmd


def _install_ntff_hook_shim():
    """antenv in this image lacks axon_hooks; bass_utils imports it whenever
    tracing is requested (including via BASS_TRACE env). Provide the
    documented ctypes-based hook so that path works instead of crashing."""
    try:
        import antenv.axon_hooks  # noqa: F401
        return
    except ImportError:
        pass
    import types, contextlib, ctypes, os

    so_path = "/opt/axon/libaxon_pjrt.so"
    hook = None
    if os.path.exists(so_path):
        try:
            lib = ctypes.CDLL(so_path)
            if hasattr(lib, "axon_start_nrt_profile"):
                lib.axon_start_nrt_profile.argtypes = [
                    ctypes.POINTER(ctypes.c_int64), ctypes.c_size_t]
                lib.axon_start_nrt_profile.restype = ctypes.c_int64
                lib.axon_stop_nrt_profile.argtypes = [ctypes.c_char_p]
                lib.axon_stop_nrt_profile.restype = ctypes.c_int64

                @contextlib.contextmanager
                def _hook(output_dir, device_ids):
                    import jax
                    jax.devices()
                    if device_ids:
                        ids = (ctypes.c_int64 * len(device_ids))(*device_ids)
                        rc = lib.axon_start_nrt_profile(ids, len(device_ids))
                    else:
                        rc = lib.axon_start_nrt_profile(None, 0)
                    if rc != 0:
                        raise RuntimeError(f"axon_start_nrt_profile rc={rc}")
                    try:
                        yield
                    finally:
                        n = lib.axon_stop_nrt_profile(str(output_dir).encode())
                        print(f"ntff profile: {n} file(s) in {output_dir}")

                hook = _hook
        except OSError:
            pass

    try:
        import antenv
    except ImportError:
        return
    m = types.ModuleType("antenv.axon_hooks")
    m.get_axon_ntff_profile_hook = (lambda h: (lambda: h))(hook)
    m.set_axon_ntff_profile_hook = lambda h: None
    sys.modules["antenv.axon_hooks"] = m
    antenv.axon_hooks = m


_install_ntff_hook_shim()

B, I, O, NUM, K = 512, 512, 512, 8, 3
NPLANES = 7
O_SPLIT, B_SPLIT = 4, 2
OQ = O // O_SPLIT    # 128 out dims per core
BH = B // B_SPLIT    # 256 batch rows per core
ICHUNKS = I // 128   # 4 partition chunks of the in_dim
FREE = ICHUNKS * BH  # 1024: feature-plane free dim (i-chunks stacked)
NCORES = O_SPLIT * B_SPLIT
NCHUNKS = NPLANES * ICHUNKS + 1   # 28 plane chunks + 1 bias chunk = 29
N_WARMUP = 6                      # dummy PE warm-up matmuls (N=512 each)

# matmul issue order = plane readiness order; bias chunk early (ready with
# its DMA piece).  Chunk ids: plane p chunk ic -> p*ICHUNKS+ic, bias -> 28.
MM_ORDER = (
    [0 * ICHUNKS + ic for ic in range(ICHUNKS)]       # P0 t
    + [1 * ICHUNKS + ic for ic in range(ICHUNKS)]     # P1 t^2
    + [28]                                            # bias x ones
    + [2 * ICHUNKS + ic for ic in range(ICHUNKS)]     # P2 t^3
    + [3 * ICHUNKS + ic for ic in range(ICHUNKS)]     # P3 |t^3|
    + [4 * ICHUNKS + ic for ic in range(ICHUNKS)]     # P4 relu(t+1)^3
    + [5 * ICHUNKS + ic for ic in range(ICHUNKS)]     # P5 relu(t-1)^3
    + [6 * ICHUNKS + ic for ic in range(ICHUNKS)]     # P6 silu
)
# DMA piece boundaries, in MM_ORDER positions (pieces of the w tensor)
W_PIECES = [9, 17, NCHUNKS]   # [P0,P1,bias] / [P2,P3] / [P4,P6,P5]

F32 = mybir.dt.float32
F16 = mybir.dt.float16


def _basis_coeffs():
    """Exact expansion of basis_k (k=0..NUM+K-1) in the phi basis.

    basis_k(x) = N(t - k) with N the cardinal cubic B-spline
    N(s) = sum_j (-1)^j C(4,j)/6 * relu(s-j)^3.  For t in [7,11) the knots
    at p <= 7 are always active (pure cubics -> poly part around t'=t-9)
    and knots p in {8,9,10} stay as relu kinks; p >= 11 never activates.
    Returns C (7, NUM+K): rows = [1, t', t'^2, t'^3, r8^3, r9^3, r10^3].
    """
    from math import comb

    nb = NUM + K
    C = np.zeros((7, nb))
    for k in range(nb):
        for j in range(5):
            w = ((-1) ** j) * comb(4, j) / 6.0
            p = k + j                      # knot index: relu(t - p)^3
            if p >= 11:
                continue
            if p <= 7:
                c = 9.0 - p
                C[0, k] += w * c ** 3
                C[1, k] += w * 3 * c ** 2
                C[2, k] += w * 3 * c
                C[3, k] += w
            else:
                C[4 + (p - 8), k] += w
    return C


def _fold_weights(grid, coef, scale_base, scale_sp, mask):
    g0 = float(grid[0, 0])
    h = float(grid[0, 1]) - g0
    C = _basis_coeffs()                                        # (7, 11)
    A = (mask.astype(np.float64) * scale_sp.astype(np.float64))[:, :, None] \
        * coef.astype(np.float64)                              # (I, O, 11)
    Wf = np.einsum("fk,iok->fio", C[1:7], A)   # rows: t,t2,t3,r8,r9,r10
    W_silu = (mask.astype(np.float64) * scale_base.astype(np.float64))[None]
    # re-express relu(t)^3 = (t^3 + |t^3|)/2 -> planes [t3, |t3|]
    W_all = np.stack([
        Wf[0], Wf[1], Wf[2] + Wf[4] / 2, Wf[4] / 2, Wf[3], Wf[5], W_silu[0],
    ], axis=0)                                                 # (7, I, O)
    bias = np.einsum("k,iok->o", C[0], A)                      # (O,)
    a1 = 1.0 / h                                               # t = a1*x + a0
    a0 = -g0 / h - 9.0
    return W_all, bias, a1, a0


def _build_nc(a1, a0):
    AF = mybir.ActivationFunctionType
    AO = mybir.AluOpType

    nc = bacc.Bacc("TRN2", target_bir_lowering=False, debug=False)
    xt_d = nc.dram_tensor("xt", [128, FREE], F16, kind="ExternalInput").ap()
    w_d = nc.dram_tensor("w", [128, NCHUNKS * 128], F16, kind="ExternalInput").ap()
    o_d = nc.dram_tensor("out", [128, BH], F16, kind="ExternalOutput").ap()

    with tile.TileContext(nc) as tc:
        with (
            tc.tile_pool(name="main", bufs=1) as pool,
            tc.tile_pool(name="ps", bufs=1, space=bass.MemorySpace.PSUM) as pp,
        ):
            # ---- input DMAs, all on the SP HWDGE ring (FIFO per ring) ----
            xs = pool.tile([128, FREE], F16, tag="xs")
            nc.sync.dma_start(xs[:], xt_d[:])
            w_sb = pool.tile([128, NCHUNKS * 128], F16, tag="w")
            lo = 0
            for hi in W_PIECES:
                nc.sync.dma_start(
                    w_sb[:, lo * 128:hi * 128], w_d[:, lo * 128:hi * 128]
                )
                lo = hi

            # ---- ones plane (bias matmul rhs + PE warm-up operand) ----
            ones = pool.tile([128, 512], F16, tag="ones")
            nc.vector.memset(ones[:], 1.0)

            # force the ACT table load to the front: tiny activation with
            # no DMA dependency (reads the memset ones tile)
            dummy_act = pool.tile([128, 1], F16, tag="dummy_act")
            nc.scalar.activation(dummy_act[:], ones[:, 0:1], AF.Square)

            acc = pp.tile([128, 512], F32, tag="acc")
            # PE warm-up: keep the HAM activity window busy during the DMA
            # wait so the real matmuls run at 2.4 GHz.  Results land in acc
            # and are discarded by the first real matmul's start=True.
            for wi in range(N_WARMUP):
                nc.tensor.matmul(
                    acc[:, 0:512], ones[:, 0:128], ones[:, 0:512],
                    start=True, stop=True,
                )

            # ---- feature planes ----
            planes = [
                pool.tile([128, FREE], F16, tag=f"pl{j}", name=f"pl{j}")
                for j in range(NPLANES)
            ]
            tp, p2, p3, pabs, f8, f10, sil = planes
            s8 = pool.tile([128, FREE], F16, tag="s8")
            s10 = pool.tile([128, FREE], F16, tag="s10")
            a8 = pool.tile([128, FREE], F16, tag="a8")
            a10 = pool.tile([128, FREE], F16, tag="a10")

            # ACT: squares + silu (bias must be an AP column; scale is an
            # immediate).  One table set covers Square+Silu+Copy.
            b2c = pool.tile([128, 1], F32, tag="b2c")
            b8c = pool.tile([128, 1], F32, tag="b8c")
            b10c = pool.tile([128, 1], F32, tag="b10c")
            nc.vector.memset(b2c[:], a0)
            nc.vector.memset(b8c[:], a0 + 1.0)
            nc.vector.memset(b10c[:], a0 - 1.0)
            nc.scalar.activation(p2[:], xs[:], AF.Square, bias=b2c[:], scale=a1)
            nc.scalar.activation(s8[:], xs[:], AF.Square, bias=b8c[:], scale=a1)
            nc.scalar.activation(s10[:], xs[:], AF.Square, bias=b10c[:], scale=a1)

            # DVE: t, relus, cubes
            nc.vector.tensor_scalar(tp[:], xs[:], a1, a0, AO.mult, AO.add)
            nc.vector.tensor_scalar(a8[:], tp[:], 1.0, 0.0, AO.add, AO.max)
            nc.vector.tensor_scalar(a10[:], tp[:], -1.0, 0.0, AO.add, AO.max)
            nc.vector.tensor_mul(p3[:], p2[:], tp[:])
            nc.vector.tensor_mul(f8[:], s8[:], a8[:])
            nc.vector.tensor_mul(f10[:], s10[:], a10[:])

            # |t^3| and silu on ACT (sil last: it is the last matmul group)
            nc.scalar.activation(pabs[:], p3[:], AF.Abs)
            nc.scalar.activation(sil[:], xs[:], AF.Silu)

            # ---- 29 accumulated matmuls in readiness order ----
            n = len(MM_ORDER)
            for pos, c in enumerate(MM_ORDER):
                if c == NCHUNKS - 1:
                    rhs = ones[:, 0:BH]
                else:
                    f, ic = divmod(c, ICHUNKS)
                    rhs = planes[f][:, ic * BH:(ic + 1) * BH]
                nc.tensor.matmul(
                    acc[:, 0:BH],
                    w_sb[:, pos * 128:(pos + 1) * 128],
                    rhs,
                    start=(pos == 0),
                    stop=(pos == n - 1),
                )

            # ---- PSUM -> SBUF on ACT (sits closer to PSUM), fp16 out ----
            outs = pool.tile([128, BH], F16, tag="outs")
            nc.scalar.activation(outs[:], acc[:, 0:BH], AF.Copy)
            nc.sync.dma_start(o_d[:], outs[:])

    nc.compile()
    return nc


def _make_in_maps(x, W_all, bias):
    """Slice + layout-swizzle the folded weights and x for the 8 cores."""
    in_maps = []
    for c in range(NCORES):
        oq, bh = c // B_SPLIT, c % B_SPLIT
        xs = x[bh * BH:(bh + 1) * BH, :]                       # (BH, I)
        xt = np.ascontiguousarray(
            xs.T.reshape(ICHUNKS, 128, BH).transpose(1, 0, 2).reshape(128, FREE)
        ).astype(np.float16)
        Wq = W_all[:, :, oq * OQ:(oq + 1) * OQ]                # (7, I, OQ)
        wc = Wq.reshape(NPLANES, ICHUNKS, 128, OQ)             # [f, ic, 128, OQ]
        bias_chunk = np.broadcast_to(
            bias[oq * OQ:(oq + 1) * OQ] / 128.0, (128, OQ)
        )
        w = np.empty((128, NCHUNKS * 128), np.float16)
        for pos, ch in enumerate(MM_ORDER):
            if ch == NCHUNKS - 1:
                w[:, pos * 128:(pos + 1) * 128] = bias_chunk
            else:
                f, ic = divmod(ch, ICHUNKS)
                w[:, pos * 128:(pos + 1) * 128] = wc[f, ic]
        in_maps.append({"xt": xt, "w": np.ascontiguousarray(w)})
    return in_maps


def _assemble(results):
    full = np.empty((B, O), np.float32)
    for c in range(NCORES):
        oq, bh = c // B_SPLIT, c % B_SPLIT
        full[bh * BH:(bh + 1) * BH, oq * OQ:(oq + 1) * OQ] = (
            results[c]["out"].astype(np.float32).T
        )
    return full


_CACHED = {}


def _get_nc(a1, a0):
    key = (a1, a0)
    if key not in _CACHED:
        _CACHED[key] = _build_nc(a1, a0)
    return _CACHED[key]


def kernel(x, grid, coef, scale_base, scale_sp, mask, _run_kwargs=None):
    x = np.asarray(x)
    W_all, bias, a1, a0 = _fold_weights(
        np.asarray(grid), np.asarray(coef), np.asarray(scale_base),
        np.asarray(scale_sp), np.asarray(mask)
    )
    nc = _get_nc(a1, a0)
    in_maps = _make_in_maps(x, W_all, bias)
    res = run_bass_kernel_spmd(
        nc, in_maps, core_ids=list(range(NCORES)), **(_run_kwargs or {})
    )
    out = _assemble(res.results)
    if _run_kwargs:
        kernel.last_result = res
    return out
